# revision 1
# baseline (speedup 1.0000x reference)
"""DiT block kernel for TRN2, 8 NeuronCores.

Sharding: DP=4 over batch x TP=2 over heads (Megatron style).
Core c: batch b=c//2, half hf=c%2 (8 of 16 heads, 2048/4096 MLP cols, 512/1024
rows of the row-parallel weights).

Device layout is feature-major: activations [feature(partitions), token(free)], so
every matmul chains without transposes: outT = W.T @ actT with lhsT=W natural layout.
v^T (lhsT of attn@v) is produced by swapping matmul operands. Softmax runs without
max-subtraction (scores are O(1)); denominators come from an augmented ones-column in
v^T (row 64 of the [65, tok] attention output).

Dtypes: trunk matmuls in float32r (TF32-like, 1 cyc/row, rel err ~1.5e-4); the
attention block (q/k/v/P/attn-out) and proj/out/fc2 weights in bf16. PSUM fp32.

Comms: 3 pair AllReduces (adaLN mod, self-attn proj+residual, cross-attn out+residual);
the fc2 reduce is folded into the output (host adds the two partials).

Weights stream through SBUF in M-blocks: slot [128, kc, mblk], one DMA per k-slice.
"""
import sys
import numpy as np

sys.path.insert(0, "/opt/trn_rl_repo")

import ml_dtypes
import concourse.bass as bass
import concourse.mybir as mybir
import concourse.tile as tile
from concourse import bacc
from concourse.bass_utils import run_bass_kernel_spmd

FP32 = mybir.dt.float32
FP32R = mybir.dt.float32r
BF16 = mybir.dt.bfloat16
AF = mybir.ActivationFunctionType
ALU = mybir.AluOpType

B, N, D, H, TD, TL = 4, 1024, 1024, 16, 768, 77
HD = 64
EPS = 1e-6
HL = 8
DL = 512
FFL = 2048
T = 1024
TLP = 80    # ctx tokens padded to even (fp32r ISA: innermost counts must be even)
NCH = D // 128
REPLICA_GROUPS = [[0, 1], [2, 3], [4, 5], [6, 7]]

# SBUF knobs (KB/partition noted at default)
P_BUFS = 7        # exp(P) tiles  (bf16 [128,1024] = 2KB each)
WST_BUFS = 2      # fp32r weight blocks (8KB each)
WST2_BUFS = 2     # bf16 weight blocks (8KB each)
XSQ_BUFS = 1
SCR_BUFS = 2


def _declare(nc):
    d = {}

    def inp(name, shape, dt):
        d[name] = nc.dram_tensor(name, list(shape), dt, kind="ExternalInput").ap()

    inp("xT", (D, T), FP32R)
    inp("cT", (128, 4), FP32)
    inp("teT", (TD, TLP), FP32R)
    inp("w_ada", (12, 128, 4, 512), FP32R)
    inp("b_ada", (1, 6 * D), FP32R)
    inp("w_qkv", (6, 128, 8, 256), FP32R)
    inp("b_qkT", (128, 8), FP32)
    inp("b_v", (1, DL), FP32)
    inp("w_proj", (2, 128, 4, 512), BF16)
    inp("b_projT", (128, 8), FP32)
    inp("w_ctx", (4, 128, 6, 256), FP32R)
    inp("b_ctxT", (128, 8), FP32)
    inp("w_q", (2, 128, 8, 256), FP32R)
    inp("w_k", (2, 128, 8, 256), FP32R)
    inp("w_v", (2, 128, 8, 256), FP32R)
    inp("w_out", (2, 128, 4, 512), BF16)
    inp("b_outT", (128, 8), FP32)
    inp("w_fc1", (8, 128, 8, 256), FP32R)
    inp("b_fc1T", (128, 16), FP32)
    inp("w_fc2", (4, 128, 16, 256), BF16)
    inp("b_fc2T", (128, 8), FP32)
    inp("ones_r", (128, 128), FP32R)
    inp("ones_b", (128, 8), BF16)
    inp("maskT", (128, 1), FP32)
    d["out_xT"] = nc.dram_tensor("out_xT", [D, T], FP32R, kind="ExternalOutput").ap()
    return d


def _emit(tc, io, pools, nocc=False):
    nc = tc.nc
    sb = pools["sb"]
    xp, hp, qkp, vtp, atp, pp = (pools[k] for k in ("xp", "hp", "qkp", "vtp", "atp", "pp"))
    wst, wst2, hidp, xsqp, scr, rcpp = (pools[k] for k in
                                        ("wst", "wst2", "hidp", "xsqp", "scr", "rcpp"))
    vecp = pools["vecp"]
    ps_mm, ps_aux = pools["ps_mm"], pools["ps_aux"]
    dram = pools["dram"]

    ones = sb.tile([128, 128], FP32R, tag="ones")
    nc.sync.dma_start(out=ones, in_=io["ones_r"])
    ones_b = sb.tile([128, 8], BF16, tag="ones_b")
    nc.sync.dma_start(out=ones_b, in_=io["ones_b"])
    mask = sb.tile([128, 1], FP32, tag="mask")
    nc.sync.dma_start(out=mask, in_=io["maskT"])

    def load_wblock(src_ap, dt, bi, pool):
        """Load pre-tiled weight block bi: host layout [nb, 128, kc, mblk] ->
        one fully-contiguous-per-partition DMA."""
        _, _, kc, mblk = src_ap.shape
        wt = pool.tile([128, kc, mblk], dt, tag="w")
        nc.sync.dma_start(out=wt, in_=src_ap[bi])
        return wt

    # ---------------- Stage 0: adaLN ----------------
    cT = sb.tile([128, 4], FP32, tag="cT")
    nc.sync.dma_start(out=cT, in_=io["cT"])
    cs = sb.tile([128, 4], FP32R, tag="cs")
    nc.scalar.activation(cs, cT, AF.Silu)

    ar_mod_in = dram.tile([1, 6 * D], FP32, tag="armod_i")
    ar_mod_out = dram.tile([1, 6 * D], FP32, tag="armod_o")

    for mb in range(12):   # w_ada rhs-blocks [512, 512]
        wt = load_wblock(io["w_ada"], FP32R, mb, wst)
        b_sl = sb.tile([1, 512], FP32R, tag="b_ada_sl", name=f"bada_{mb}")
        nc.sync.dma_start(out=b_sl, in_=io["b_ada"][:, 512 * mb:512 * (mb + 1)])
        pm = ps_aux.tile([1, 512], FP32, tag="aux")
        for k in range(4):
            nc.tensor.matmul(pm, cs[:, k:k + 1], wt[:, k, :], start=(k == 0), stop=False)
        nc.tensor.matmul(pm, ones[0:1, 0:1], b_sl, start=False, stop=True)
        mp = vecp.tile([1, 512], FP32, tag="tmpv", name=f"modp_{mb}")
        nc.scalar.copy(mp, pm)
        nc.sync.dma_start(out=ar_mod_in[:, 512 * mb:512 * (mb + 1)], in_=mp)
    # Split mod AllReduce: the msa groups (cols 0:2048) gate LN1 and reduce after
    # only 4 of 12 ada blocks; the rest reduces later, off the critical path.
    # (nocc-sim charges +4us for the extra DMA op but cannot see the collective
    # latency this hides on real HW.)
    modT = sb.tile([128, 6, 8], FP32, tag="modT")
    ksc = sb.tile([128, 3, 8], FP32, tag="ksc")
    mod_view = ar_mod_out.rearrange("o (g j p) -> (o p) g j", p=128, g=6)
    for (c0, c1, g0, g1) in [(0, 2048, 0, 2), (2048, 6144, 2, 6)]:
        if nocc:
            nc.sync.dma_start(out=ar_mod_out[:, c0:c1], in_=ar_mod_in[:, c0:c1])
        else:
            nc.gpsimd.collective_compute(
                "AllReduce", ALU.add, replica_groups=REPLICA_GROUPS,
                ins=[ar_mod_in[:, c0:c1].opt()], outs=[ar_mod_out[:, c0:c1].opt()])
        nc.sync.dma_start(out=modT[:, g0:g1, :], in_=mod_view[:, g0:g1, :])
        for i in range(g0 // 2, g1 // 2):
            nc.vector.tensor_scalar(ksc[:, i, :], modT[:, 2 * i + 1, :], 1.0, None,
                                    op0=ALU.add)
    eps_t = sb.tile([1, 1], FP32, tag="eps")
    nc.vector.memset(eps_t, EPS)

    # ---------------- LN + modulate ----------------
    def layer_norm_mod(x_tiles, g_sh, g_sc):
        sum_ps = ps_aux.tile([1, T], FP32, tag="aux")
        sq_ps = ps_aux.tile([1, T], FP32, tag="aux")
        for j in range(NCH):
            xsq = xsqp.tile([128, T], FP32R, tag="xsq")
            nc.vector.tensor_tensor(xsq, x_tiles[j], x_tiles[j], op=ALU.mult)
            for nb in range(2):
                s = slice(512 * nb, 512 * (nb + 1))
                nc.tensor.matmul(sum_ps[:, s], ones[:, 0:1], x_tiles[j][:, s],
                                 start=(j == 0), stop=(j == NCH - 1), skip_group_check=True)
                nc.tensor.matmul(sq_ps[:, s], ones[:, 1:2], xsq[:, s],
                                 start=(j == 0), stop=(j == NCH - 1), skip_group_check=True)
        mu = vecp.tile([1, T], FP32R, tag="mu")
        nc.scalar.activation(mu, sum_ps, AF.Copy, scale=1.0 / D)
        musq = vecp.tile([1, T], FP32, tag="tmpv")
        nc.vector.tensor_tensor(musq, mu, mu, op=ALU.mult)
        var = vecp.tile([1, T], FP32, tag="tmpv2")
        nc.vector.scalar_tensor_tensor(var, sq_ps, 1.0 / D, musq,
                                       op0=ALU.mult, op1=ALU.subtract)
        sig = vecp.tile([1, T], FP32, tag="tmpv")
        nc.scalar.activation(sig, var, AF.Sqrt, bias=eps_t)
        rsig = vecp.tile([1, T], FP32R, tag="rsig")
        with nc.allow_low_precision(reason="fp32r rsig feeds fp32r broadcast matmul"):
            nc.vector.reciprocal(rsig, sig)
        mu_b = ps_aux.tile([128, T], FP32, tag="aux")
        rs_b = ps_aux.tile([128, T], FP32, tag="aux")
        for nb in range(2):
            s = slice(512 * nb, 512 * (nb + 1))
            nc.tensor.matmul(mu_b[:, s], ones[0:1, :], mu[:, s], start=True, stop=True)
            nc.tensor.matmul(rs_b[:, s], ones[0:1, :], rsig[:, s], start=True, stop=True)
        h_tiles = []
        for j in range(NCH):
            t1 = scr.tile([128, T], FP32, tag="t1")
            nc.vector.tensor_tensor(t1, x_tiles[j], mu_b, op=ALU.subtract)
            nc.vector.scalar_tensor_tensor(t1, t1, ksc[:, g_sc, j:j + 1], rs_b,
                                           op0=ALU.mult, op1=ALU.mult)
            h = hp.tile([128, T], FP32R, tag="h")
            nc.vector.tensor_scalar(h, t1, modT[:, g_sh, j:j + 1], None, op0=ALU.add)
            h_tiles.append(h)
        return h_tiles

    # ---------------- Stage 1: x + LN1 ----------------
    x1 = []
    for j in range(NCH):
        xt = xp.tile([128, T], FP32R, tag="x")
        nc.sync.dma_start(out=xt, in_=io["xT"][128 * j:128 * (j + 1), :])
        x1.append(xt)
    h1 = layer_norm_mod(x1, g_sh=0, g_sc=0)

    # ---------------- Stage 2: qkv + vT ----------------
    b_qkT = sb.tile([128, 8], FP32, tag="b_qkT")
    nc.sync.dma_start(out=b_qkT, in_=io["b_qkT"])
    bv_b = sb.tile([128, DL], FP32, tag="bv")
    bv_src = io["b_v"]
    nc.sync.dma_start(out=bv_b, in_=bass.AP(tensor=bv_src.tensor, offset=bv_src.offset,
                                            ap=[[0, 128]] + list(bv_src.ap[1:])))
    # q,k: feature-major out, 8 m-chunks (0..3 q, 4..7 k) via 4 M-blocks of 256
    qkT = []
    for mb in range(4):
        wt = load_wblock(io["w_qkv"], FP32R, mb, wst)
        for mm in range(2):
            m = 2 * mb + mm
            pm = ps_mm.tile([128, T], FP32, tag="mm")
            for nb in range(2):
                s = slice(512 * nb, 512 * (nb + 1))
                for k in range(NCH):
                    nc.tensor.matmul(pm[:, s], wt[:, k, 128 * mm:128 * (mm + 1)],
                                     h1[k][:, s], start=(k == 0), stop=(k == NCH - 1))
            qk = qkp.tile([128, T], BF16, tag="qk")
            nc.vector.tensor_scalar(qk, pm, b_qkT[:, m:m + 1], None, op0=ALU.add)
            qkT.append(qk)
    # vT: token-major out with ones-augmented head columns
    wv_blk = [load_wblock(io["w_qkv"], FP32R, 4 + g, wst)
              for g in range(2)]
    vT = []
    for m in range(NCH):
        pm = ps_aux.tile([128, DL], FP32, tag="aux")
        for g in range(2):
            for k in range(NCH):
                nc.tensor.matmul(pm[:, 256 * g:256 * (g + 1)],
                                 h1[k][:, 128 * m:128 * (m + 1)], wv_blk[g][:, k, :],
                                 start=(k == 0), stop=(k == NCH - 1))
        vt = vtp.tile([128, HL, HD + 1], BF16, tag="vt")
        nc.vector.tensor_tensor(vt[:, :, 0:HD],
                                pm.rearrange("p (a b) -> p a b", a=HL),
                                bv_b.rearrange("p (a b) -> p a b", a=HL), op=ALU.add)
        nc.sync.dma_start(out=vt[:, :, HD:HD + 1],
                          in_=io["ones_b"].rearrange("p (j o) -> p j o", o=1))
        vT.append(vt)

    # ---------------- attention ----------------
    def attention(q_tiles, k_tiles, v_tiles, ktoks):
        nchunk = (ktoks + 127) // 128
        at_tiles = [atp.tile([128, T], BF16, tag="at", name=f"at_{i}") for i in range(4)]
        for h in range(HL):
            ti, off = h // 2, 64 * (h % 2)
            q_ap = q_tiles[ti][off:off + 64, :]
            pts = []
            for m in range(nchunk):
                mk = min(128, ktoks - 128 * m)
                ps_sc = ps_mm.tile([128, T], FP32, tag="mm")
                k_ap = k_tiles[ti][off:off + 64, 128 * m:128 * m + mk]
                for nb in range(2):
                    s = slice(512 * nb, 512 * (nb + 1))
                    nc.tensor.matmul(ps_sc[:mk, s], k_ap, q_ap[:, s], start=True, stop=True)
                pt = pp.tile([128, T], BF16, tag="p")
                nc.scalar.activation(pt[:mk, :], ps_sc[:mk, :], AF.Exp,
                                     scale=float(HD) ** -0.5)
                pts.append((pt, mk))
            po = ps_aux.tile([65, T], FP32, tag="aux")
            for m, (pt, mk) in enumerate(pts):
                for nb in range(2):
                    s = slice(512 * nb, 512 * (nb + 1))
                    nc.tensor.matmul(po[:, s], v_tiles[m][:mk, h, :], pt[:mk, s],
                                     start=(m == 0), stop=(m == nchunk - 1),
                                     skip_group_check=True)
            rcp = rcpp.tile([1, T], FP32R, tag="rcp")
            with nc.allow_low_precision(reason="fp32r rcp feeds fp32r broadcast matmul"):
                nc.vector.reciprocal(rcp, po[64:65, :])
            pb = ps_aux.tile([64, T], FP32, tag="aux")
            for nb in range(2):
                s = slice(512 * nb, 512 * (nb + 1))
                nc.tensor.matmul(pb[:, s], ones[0:1, 0:64], rcp[:, s], start=True, stop=True)
            rc_sb = rcpp.tile([64, T], FP32, tag="rc_sb")
            nc.vector.tensor_copy(rc_sb, pb)
            nc.vector.tensor_tensor(at_tiles[ti][off:off + 64, :], po[0:64, :], rc_sb,
                                    op=ALU.mult)
        return at_tiles

    at1 = attention(qkT[0:4], qkT[4:8], vT, T)

    # ---------------- row-parallel + fold residual + AllReduce ----------------
    def row_parallel_reduce(w_name, bT_name, act_tiles, x_tiles, nk, ar_tag):
        bT = sb.tile([128, 8], FP32, tag=bT_name)
        nc.sync.dma_start(out=bT, in_=io[bT_name])
        ar_in = dram.tile([D, T], FP32, tag=ar_tag + "_i")
        ar_out = dram.tile([D, T], FP32, tag=ar_tag + "_o")
        for mb in range(2):
            wt = load_wblock(io[w_name], BF16, mb, wst2)
            for mm in range(4):
                m = 4 * mb + mm
                pm = ps_mm.tile([128, T], FP32, tag="mm")
                for nb in range(2):
                    s = slice(512 * nb, 512 * (nb + 1))
                    for k in range(nk):
                        nc.tensor.matmul(pm[:, s], wt[:, k, 128 * mm:128 * (mm + 1)],
                                         act_tiles[k][:, s], start=(k == 0),
                                         stop=(k == nk - 1))
                e2 = scr.tile([128, T], FP32, tag="t1")
                nc.vector.tensor_scalar(e2, pm, bT[:, m:m + 1], None, op0=ALU.add)
                nc.vector.scalar_tensor_tensor(e2, x_tiles[m], mask, e2,
                                               op0=ALU.mult, op1=ALU.add)
                nc.sync.dma_start(out=ar_in[128 * m:128 * (m + 1), :], in_=e2)
        for hh in range(2):
            sl = slice(hh * D // 2, (hh + 1) * D // 2)
            if nocc:
                nc.sync.dma_start(out=ar_out[sl, :], in_=ar_in[sl, :])
            else:
                nc.gpsimd.collective_compute(
                    "AllReduce", ALU.add, replica_groups=REPLICA_GROUPS,
                    ins=[ar_in[sl, :].opt()], outs=[ar_out[sl, :].opt()])
        newx = []
        for m in range(NCH):
            xt = xp.tile([128, T], FP32R, tag="x")
            nc.gpsimd.dma_start(out=xt, in_=ar_out[128 * m:128 * (m + 1), :])
            newx.append(xt)
        return newx

    x2 = row_parallel_reduce("w_proj", "b_projT", at1, x1, 4, "arp")

    # ---------------- Stage 4: cross-attention ----------------
    teT = []
    for k in range(TD // 128):
        tt = pools["tep"].tile([128, TLP], FP32R, tag="teT")
        nc.sync.dma_start(out=tt, in_=io["teT"][128 * k:128 * (k + 1), :])
        teT.append(tt)
    b_ctxT = sb.tile([128, 8], FP32, tag="b_ctxT")
    nc.sync.dma_start(out=b_ctxT, in_=io["b_ctxT"])
    ctxT = []
    for mb in range(4):
        wt = load_wblock(io["w_ctx"], FP32R, mb, wst)
        for mm in range(2):
            m = 2 * mb + mm
            pm = ps_aux.tile([128, TLP], FP32, tag="aux")
            for k in range(TD // 128):
                nc.tensor.matmul(pm, wt[:, k, 128 * mm:128 * (mm + 1)], teT[k],
                                 start=(k == 0), stop=(k == TD // 128 - 1))
            ct = pools["ctxp"].tile([128, TLP], FP32R, tag="ctxT")
            nc.vector.tensor_scalar(ct, pm, b_ctxT[:, m:m + 1], None, op0=ALU.add)
            ctxT.append(ct)

    h2 = layer_norm_mod(x2, g_sh=2, g_sc=1)

    q2T = []
    for mb in range(2):
        wt = load_wblock(io["w_q"], FP32R, mb, wst)
        for mm in range(2):
            m = 2 * mb + mm
            pm = ps_mm.tile([128, T], FP32, tag="mm")
            for nb in range(2):
                s = slice(512 * nb, 512 * (nb + 1))
                for k in range(NCH):
                    nc.tensor.matmul(pm[:, s], wt[:, k, 128 * mm:128 * (mm + 1)],
                                     h2[k][:, s], start=(k == 0), stop=(k == NCH - 1))
            qt = qkp.tile([128, T], BF16, tag="qk")
            nc.vector.tensor_copy(qt, pm)
            q2T.append(qt)
    kcT = []
    for mb in range(2):
        wt = load_wblock(io["w_k"], FP32R, mb, wst)
        for mm in range(2):
            m = 2 * mb + mm
            pm = ps_aux.tile([128, TLP], FP32, tag="aux")
            for k in range(NCH):
                nc.tensor.matmul(pm, wt[:, k, 128 * mm:128 * (mm + 1)], ctxT[k],
                                 start=(k == 0), stop=(k == NCH - 1))
            kt = qkp.tile([128, T], BF16, tag="qk")
            nc.vector.tensor_copy(kt[:, 0:TLP], pm)
            kcT.append(kt)
    # vc: [77, HL, 65]
    vc_blk = [load_wblock(io["w_v"], FP32R, g, wst) for g in range(2)]
    pv = ps_aux.tile([TLP, DL], FP32, tag="aux")
    for g in range(2):
        for k in range(NCH):
            nc.tensor.matmul(pv[:, 256 * g:256 * (g + 1)], ctxT[k],
                             vc_blk[g][:, k, :], start=(k == 0), stop=(k == NCH - 1))
    vc = vtp.tile([128, HL, HD + 1], BF16, tag="vt")
    nc.vector.memset(vc, 0.0)
    nc.vector.tensor_copy(vc[0:TL, :, 0:HD], pv[0:TL].rearrange("p (a b) -> p a b", a=HL))
    nc.sync.dma_start(out=vc[0:TL, :, HD:HD + 1],
                      in_=io["ones_b"][0:TL, :].rearrange("p (j o) -> p j o", o=1))

    at2 = attention(q2T, kcT, [vc], TLP)
    x3 = row_parallel_reduce("w_out", "b_outT", at2, x2, 4, "aro")

    # ---------------- Stage 5: MLP ----------------
    h3 = layer_norm_mod(x3, g_sh=4, g_sc=2)
    b_fc1T = sb.tile([128, 16], FP32, tag="b_fc1T")
    nc.sync.dma_start(out=b_fc1T, in_=io["b_fc1T"])
    b_fc2T = sb.tile([128, 8], FP32, tag="b_fc2T")
    nc.sync.dma_start(out=b_fc2T, in_=io["b_fc2T"])
    for tb in range(2):
        s = slice(512 * tb, 512 * (tb + 1))
        hid = []
        for mb in range(8):
            wt = load_wblock(io["w_fc1"], FP32R, mb, wst)
            for mm in range(2):
                m = 2 * mb + mm
                pm = ps_aux.tile([128, 512], FP32, tag="aux")
                for k in range(NCH):
                    nc.tensor.matmul(pm, wt[:, k, 128 * mm:128 * (mm + 1)], h3[k][:, s],
                                     start=(k == 0), stop=(k == NCH - 1))
                ht = hidp.tile([128, 512], BF16, tag="hid")
                nc.scalar.activation(ht, pm, AF.Gelu, bias=b_fc1T[:, m:m + 1])
                hid.append(ht)
        for mb in range(4):
            wt2 = load_wblock(io["w_fc2"], BF16, mb, wst2)
            for mm in range(2):
                m = 2 * mb + mm
                pm = ps_mm.tile([128, 512], FP32, tag="mm")
                for k in range(FFL // 128):
                    nc.tensor.matmul(pm, wt2[:, k, 128 * mm:128 * (mm + 1)], hid[k],
                                     start=(k == 0), stop=(k == FFL // 128 - 1))
                ot = scr.tile([128, 512], FP32R, tag="ot")
                nc.vector.tensor_scalar(ot, pm, b_fc2T[:, m:m + 1], None, op0=ALU.add)
                nc.vector.scalar_tensor_tensor(ot, x3[m][:, s], mask, ot,
                                               op0=ALU.mult, op1=ALU.add)
                nc.sync.dma_start(out=io["out_xT"][128 * m:128 * (m + 1), s], in_=ot)


def build(nocc=False):
    nc = bacc.Bacc("TRN2", target_bir_lowering=False, debug=False,
                   num_devices=1 if nocc else 8)
    io = _declare(nc)
    with tile.TileContext(nc) as tc:
        import contextlib
        with contextlib.ExitStack() as ctx:
            def pool(name, bufs, space="SBUF"):
                return ctx.enter_context(tc.tile_pool(name=name, bufs=bufs, space=space))
            pools = {
                "sb": pool("sb", 1),
                "xp": pool("xp", 8),
                "hp": pool("hp", 8),
                "qkp": pool("qkp", 8),
                "vtp": pool("vtp", 8),
                "atp": pool("atp", 4),
                "pp": pool("pp", P_BUFS),
                "wst": pool("wst", WST_BUFS),
                "wst2": pool("wst2", WST2_BUFS),
                "hidp": pool("hidp", 16),
                "xsqp": pool("xsqp", XSQ_BUFS),
                "scr": pool("scr", SCR_BUFS),
                "rcpp": pool("rcpp", 1),
                "tep": pool("tep", 6),
                "ctxp": pool("ctxp", 8),
                "vecp": pool("vecp", 1),
                "ps_mm": pool("ps_mm", 2, "PSUM"),
                "ps_aux": pool("ps_aux", 2, "PSUM"),
                "dram": pool("dram", 1, "DRAM"),
            }
            _emit(tc, io, pools, nocc=nocc)
    nc.compile()
    return nc


def pretile(w, mblk):
    """[K, M] -> [M//mblk, 128, K//128, mblk] contiguous blocks."""
    K, M = w.shape
    kc = K // 128
    v = w.reshape(kc, 128, M // mblk, mblk).transpose(2, 1, 0, 3)
    return np.ascontiguousarray(v)


def shard_inputs(inputs):
    f32 = np.float32
    bf16 = ml_dtypes.bfloat16
    x = np.asarray(inputs["x"], f32)
    c = np.asarray(inputs["c"], f32)
    te = np.asarray(inputs["text_embed"], f32)
    W_ada, b_ada = np.asarray(inputs["W_ada"], f32), np.asarray(inputs["b_ada"], f32)
    W_qkv, b_qkv = np.asarray(inputs["W_qkv"], f32), np.asarray(inputs["b_qkv"], f32)
    W_proj, b_proj = np.asarray(inputs["W_proj"], f32), np.asarray(inputs["b_proj"], f32)
    W_ctx, b_ctx = np.asarray(inputs["W_ctx"], f32), np.asarray(inputs["b_ctx"], f32)
    W_q, W_k, W_v = (np.asarray(inputs[k], f32) for k in ("W_q", "W_k", "W_v"))
    W_out, b_out = np.asarray(inputs["W_out"], f32), np.asarray(inputs["b_out"], f32)
    W_fc1, b_fc1 = np.asarray(inputs["W_fc1"], f32), np.asarray(inputs["b_fc1"], f32)
    W_fc2, b_fc2 = np.asarray(inputs["W_fc2"], f32), np.asarray(inputs["b_fc2"], f32)

    maps = []
    for core in range(8):
        b, hf = core // 2, core % 2
        sl = slice(DL * hf, DL * (hf + 1))
        half = (lambda a: a) if hf == 0 else (lambda a: np.zeros_like(a))
        qs = slice(DL * hf, DL * (hf + 1))
        ks_ = slice(D + DL * hf, D + DL * (hf + 1))
        vs = slice(2 * D + DL * hf, 2 * D + DL * (hf + 1))
        m = {
            "xT": np.ascontiguousarray(x[b].T),
            "cT": np.ascontiguousarray(c[b, sl].reshape(4, 128).T),
            "teT": np.ascontiguousarray(np.pad(te[b].T, ((0, 0), (0, TLP - TL)))),
            "w_ada": pretile(W_ada[sl, :], 512),
            "b_ada": half(b_ada)[None, :],
            "w_qkv": pretile(np.concatenate(
                [W_qkv[:, qs], W_qkv[:, ks_], W_qkv[:, vs]], axis=1), 256),
            "b_qkT": np.ascontiguousarray(
                np.concatenate([b_qkv[qs], b_qkv[ks_]]).reshape(8, 128).T),
            "b_v": b_qkv[vs][None, :],
            "w_proj": pretile(W_proj[sl, :].astype(bf16), 512),
            "b_projT": np.ascontiguousarray(half(b_proj).reshape(8, 128).T),
            "w_ctx": pretile(W_ctx, 256),
            "b_ctxT": np.ascontiguousarray(b_ctx.reshape(8, 128).T),
            "w_q": pretile(W_q[:, sl], 256),
            "w_k": pretile(W_k[:, sl], 256),
            "w_v": pretile(W_v[:, sl], 256),
            "w_out": pretile(W_out[sl, :].astype(bf16), 512),
            "b_outT": np.ascontiguousarray(half(b_out).reshape(8, 128).T),
            "w_fc1": pretile(W_fc1[:, FFL * hf:FFL * (hf + 1)], 256),
            "b_fc1T": np.ascontiguousarray(
                b_fc1[FFL * hf:FFL * (hf + 1)].reshape(16, 128).T),
            "w_fc2": pretile(W_fc2[FFL * hf:FFL * (hf + 1), :].astype(bf16), 256),
            "b_fc2T": np.ascontiguousarray(half(b_fc2).reshape(8, 128).T),
            "ones_r": np.ones((128, 128), f32),
            "ones_b": np.ones((128, 8), bf16),
            "maskT": np.full((128, 1), 1.0 - hf, f32),
        }
        maps.append(m)
    return maps


_NC_CACHE = None


def kernel(**inputs):
    global _NC_CACHE
    if _NC_CACHE is None:
        _NC_CACHE = build()
    nc = _NC_CACHE
    in_maps = shard_inputs(inputs)
    res = run_bass_kernel_spmd(nc, in_maps, core_ids=list(range(8)))
    out = np.empty((B, N, D), np.float32)
    for b in range(B):
        p0 = res.results[2 * b]["out_xT"]
        p1 = res.results[2 * b + 1]["out_xT"]
        out[b] = (p0.astype(np.float32) + p1.astype(np.float32)).T
    return out



# revision 29
# speedup vs baseline: 1.3412x; 1.3412x over previous
"""DiT block kernel for TRN2, 8 NeuronCores.

Sharding: DP=4 over batch x TP=2 over heads. Core c: batch b=c//2, half hf=c%2.

Layout: feature-major activations [feature(part), token(free)]; all matmuls chain
as outT = W.T @ actT. Softmax without max-subtraction (scores <= ~4); denominators
via ones-column in v^T.

Dtypes: trunk x fp32r; LN stats fp32r; scores q/k and MLP bf16; fp8e4(+DoubleRow)
for qkv, q2, ctx-proj, k/v-ctx, proj, out and self-attn P@v. PSUM fp32.

adaLN is column-split 8-way (interleaved 128-chunks) + AllGather; the two
residual AllReduces carry bf16 deltas only (residual added locally after).
"""
import sys
import numpy as np

sys.path.insert(0, "/opt/trn_rl_repo")

import ml_dtypes
import concourse.bass as bass
import concourse.mybir as mybir
import concourse.tile as tile
from concourse import bacc
from concourse.bass_utils import run_bass_kernel_spmd

FP32 = mybir.dt.float32
FP32R = mybir.dt.float32r
BF16 = mybir.dt.bfloat16
FP8 = mybir.dt.float8e4
AF = mybir.ActivationFunctionType
ALU = mybir.AluOpType
DR = mybir.MatmulPerfMode.DoubleRow

B, N, D, H, TD, TL = 4, 1024, 1024, 16, 768, 77
HD = 64
EPS = 1e-6
HL = 8          # heads per core
DL = 512        # head-features per core
FFL = 2048      # MLP hidden per core
T = 1024
TLP = 80
NCH = D // 128
PAIR_GROUPS = [[0, 1], [2, 3], [4, 5], [6, 7]]
ALL_GROUP = [[0, 1, 2, 3, 4, 5, 6, 7]]
EXPB = -1.0     # exp bias: P = exp(s - 1), cancels in normalization


def _declare(nc):
    d = {}

    def inp(name, shape, dt):
        d[name] = nc.dram_tensor(name, list(shape), dt, kind="ExternalInput").ap()

    inp("xT", (D, T), FP32R)
    inp("cT", (128, 8, 4), BF16)          # c feature-major, all batches
    inp("teT", (128, 3, 2, TLP), FP8)     # text embed, DR-paired
    inp("w_ada", (6, 128, 8, 128), BF16)  # interleaved col-slice per core
    inp("b_adaT", (128, 6), FP32)
    inp("w_qk", (4, 128, 4, 2, 256), FP8)
    inp("b_qkT", (128, 8), FP32)
    inp("w_v", (2, 128, 4, 2, 256), FP8)
    inp("b_v", (1, DL), FP32)
    inp("w_proj", (2, 128, 2, 2, 512), FP8)
    inp("b_projT", (128, 8), FP32)
    inp("w_ctx", (4, 128, 3, 2, 256), FP8)
    inp("b_ctxT", (128, 8), FP32)
    inp("w_q", (2, 128, 4, 2, 256), FP8)
    inp("w_k", (2, 128, 4, 2, 256), FP8)
    inp("w_vc", (2, 128, 4, 2, 256), FP8)
    inp("w_out", (2, 128, 2, 2, 512), FP8)
    inp("b_outT", (128, 8), FP32)
    inp("w_fc1", (8, 128, 8, 256), BF16)
    inp("b_fc1T", (128, 16), FP32)
    inp("w_fc2", (4, 128, 16, 256), BF16)
    inp("b_fc2T", (128, 8), FP32)
    inp("ones_r", (128, 2), FP32R)
    inp("ones2", (34, 128), FP32R)         # block-diag per-head-pair broadcast
    inp("ones_8", (128, 8), FP8)
    inp("ones_b", (128, 8), BF16)
    d["out_xT"] = nc.dram_tensor("out_xT", [D, T], FP32R, kind="ExternalOutput").ap()
    return d


def _emit(tc, io, pools, nocc=False):
    nc = tc.nc
    sb = pools["sb"]
    xp, hp, h3p, qkp, vtp, atp, pp = (pools[k] for k in
                                      ("xp", "hp", "h3p", "qkp", "vtp", "atp", "pp"))
    wst, wst2, wst3, hidp, xsqp, scr = (pools[k] for k in
                                        ("wst", "wst2", "wst3", "hidp", "xsqp", "scr"))
    vecp, rcpp, ddp = (pools[k] for k in ("vecp", "rcpp", "ddp"))
    ps_mm, ps_aux = pools["ps_mm"], pools["ps_aux"]
    dram = pools["dram"]

    ones = sb.tile([128, 2], FP32R, tag="ones")
    nc.sync.dma_start(out=ones, in_=io["ones_r"])
    ones2 = sb.tile([34, 128], FP32R, tag="ones2")
    nc.sync.dma_start(out=ones2, in_=io["ones2"])

    def load_w(src_ap, dt, bi, pool):
        wt = pool.tile(list(src_ap.shape[1:]), dt, tag="w")
        nc.sync.dma_start(out=wt, in_=src_ap[bi])
        return wt

    # ---------------- Stage 0: adaLN (8-way col split + AllGather) ----------
    cT = sb.tile([128, 8, 4], BF16, tag="cT")
    nc.sync.dma_start(out=cT, in_=io["cT"])
    b_adaT = sb.tile([128, 6], FP32, tag="b_adaT")
    nc.sync.dma_start(out=b_adaT, in_=io["b_adaT"])
    cs = cT
    nc.scalar.activation(cs, cT, AF.Silu)

    # AllToAll: in row r = my col-slice of mod for batch r//2; out row j =
    # rank j's col-slice for MY batch. Layout core-independent -> SPMD-safe.
    ag_in = dram.tile([8, 768], FP32, tag="ag_i")
    ag_out = dram.tile([8, 768], FP32, tag="ag_o")
    ag_in_v = ag_in.rearrange("(b r) (t p) -> p t b r", p=128, b=4)
    for t in range(6):
        wt = load_w(io["w_ada"], BF16, t, pools["wadap"])
        pm = ps_aux.tile([128, 4], FP32, tag="aux")
        for k in range(8):
            nc.tensor.matmul(pm, wt[:, k, :], cs[:, k, :], start=(k == 0), stop=(k == 7))
        mp = vecp.tile([128, 4], FP32, tag="tmpv", name=f"modp_{t}")
        nc.vector.tensor_scalar(mp, pm, b_adaT[:, t:t + 1], None, op0=ALU.add)
        for r in range(2):
            nc.sync.dma_start(out=ag_in_v[:, t, :, r], in_=mp)
    if nocc:
        nc.gpsimd.dma_start(out=ag_out, in_=ag_in)
    else:
        nc.gpsimd.collective_compute(
            "AllToAll", ALU.bypass, replica_groups=ALL_GROUP,
            ins=[ag_in.opt()], outs=[ag_out.opt()])

    # mod views: modT [128, 6, 8] feature-partition; kss rows (sc, sh, ones)
    # per LN group for the S-broadcast lhsT.
    modT = sb.tile([128, 6, 8], FP32, tag="modT")
    modT_src = ag_out.rearrange("j (t p) -> p t j", p=128)
    for t in range(6):
        nc.sync.dma_start(out=modT[:, t, :], in_=modT_src[:, t, :])
    ksf = ag_out.rearrange("j (t p) -> t j p", p=128)
    kss_t = sb.tile([67, 8, 128], FP32R, tag="kss")
    for i in range(3):
        nc.gpsimd.dma_start(out=kss_t[32 * i:32 * i + 1, :, :],
                            in_=ksf[2 * i + 1:2 * i + 2])
        nc.gpsimd.dma_start(out=kss_t[32 * i + 1:32 * i + 2, :, :],
                            in_=ksf[2 * i:2 * i + 1])
        nc.vector.memset(kss_t[32 * i + 2:32 * i + 3, :, :], 1.0)
    kss = [kss_t[32 * i:32 * i + 3] for i in range(3)]
    ksc = sb.tile([128, 3, 8], FP32, tag="ksc")
    nc.vector.tensor_scalar(ksc, modT.rearrange("p (g a) j -> p g a j", a=2)[:, :, 1, :],
                            1.0, None, op0=ALU.add)
    eps_t = sb.tile([1, 1], FP32, tag="eps")
    nc.vector.memset(eps_t, EPS)
    expb_t = sb.tile([128, 1], FP32, tag="expb")
    nc.vector.memset(expb_t, EXPB)
    rhs33 = sb.tile([67, T], FP32R, tag="rhs33")
    for i in range(3):
        nc.vector.memset(rhs33[32 * i + 1:32 * i + 2, :], -1.0)

    # ---------------- LN + modulate ----------------
    # h = (x*ksc)*R - S ; R = bcast(rsig); S_j = (1+sc_j)*murs - sh_j
    def layer_norm_mod(x_tiles, grp, write_h):
        sum_ps = ps_mm.tile([1, T], FP32, tag="mm")
        sq_ps = ps_mm.tile([1, T], FP32, tag="mm")
        for j in range(NCH):
            xsq = xsqp.tile([128, T], FP32R, tag="xsq")
            nc.scalar.activation(xsq, x_tiles[j], AF.Square)
            for nb in range(2):
                s = slice(512 * nb, 512 * (nb + 1))
                nc.tensor.matmul(sum_ps[:, s], ones[:, 0:1], x_tiles[j][:, s],
                                 start=(j == 0), stop=(j == NCH - 1), skip_group_check=True)
                nc.tensor.matmul(sq_ps[:, s], ones[:, 1:2], xsq[:, s],
                                 start=(j == 0), stop=(j == NCH - 1), skip_group_check=True)
        mu = vecp.tile([1, T], FP32R, tag="mu")
        nc.scalar.activation(mu, sum_ps, AF.Copy, scale=1.0 / D)
        musq = vecp.tile([1, T], FP32, tag="tmpv")
        nc.vector.tensor_tensor(musq, mu, mu, op=ALU.mult)
        var = vecp.tile([1, T], FP32, tag="tmpv2")
        nc.vector.scalar_tensor_tensor(var, sq_ps, 1.0 / D, musq,
                                       op0=ALU.mult, op1=ALU.subtract)
        sig = vecp.tile([1, T], FP32, tag="tmpv")
        nc.scalar.activation(sig, var, AF.Sqrt, bias=eps_t)
        rsig = vecp.tile([1, T], FP32R, tag="rsig")
        with nc.allow_low_precision(reason="fp32r rsig feeds broadcast matmul"):
            nc.vector.reciprocal(rsig, sig)
        # rhs33 rows (per grp at 32*grp): [murs, -1, murs]
        r0 = 32 * grp
        nc.vector.tensor_tensor(rhs33[r0:r0 + 1, :], mu, rsig, op=ALU.mult)
        nc.vector.tensor_copy(rhs33[r0 + 2:r0 + 3, :], rhs33[r0:r0 + 1, :])
        R = ps_aux.tile([128, T], FP32, tag="aux")
        for nb in range(2):
            s = slice(512 * nb, 512 * (nb + 1))
            nc.tensor.matmul(R[:, s], ones2[0:1, :], rsig[:, s], start=True, stop=True)
        for j in range(NCH):
            S = ps_aux.tile([128, T], FP32, tag="aux", name=f"S_{grp}_{j}")
            for nb in range(2):
                s = slice(512 * nb, 512 * (nb + 1))
                nc.tensor.matmul(S[:, s], kss[grp][:, j, :],
                                 rhs33[32 * grp:32 * grp + 3, s],
                                 start=True, stop=True)
            t1 = scr.tile([128, T], FP32, tag="t1")
            nc.vector.scalar_tensor_tensor(t1, x_tiles[j], ksc[:, grp, j:j + 1], R,
                                           op0=ALU.mult, op1=ALU.mult)
            write_h(j, t1, S)

    # ---------------- Stage 1: x + LN1 ----------------
    x_tiles = []
    for j in range(NCH):
        xt = xp.tile([128, T], FP32R, tag="x")
        nc.sync.dma_start(out=xt, in_=io["xT"][128 * j:128 * (j + 1), :])
        x_tiles.append(xt)

    h1 = [hp.tile([128, 2, T], FP8, tag="h", name=f"h1_{g}") for g in range(4)]

    def write_h1(j, t1, S):
        nc.vector.tensor_tensor(h1[j // 2][:, j % 2, :], t1, S, op=ALU.subtract)

    layer_norm_mod(x_tiles, 0, write_h1)

    # ---------------- Stage 2: qkv ----------------
    b_qkT = sb.tile([128, 8], FP32, tag="b_qkT")
    nc.sync.dma_start(out=b_qkT, in_=io["b_qkT"])
    bv_b = sb.tile([128, DL], FP32, tag="bv")
    bv_src = io["b_v"]
    nc.sync.dma_start(out=bv_b, in_=bass.AP(tensor=bv_src.tensor, offset=bv_src.offset,
                                            ap=[[0, 128]] + list(bv_src.ap[1:])))
    qkT = []
    for mb in range(4):
        wt = load_w(io["w_qk"], FP8, mb, wst)
        for mm in range(2):
            m = 2 * mb + mm
            pm = ps_mm.tile([128, T], FP32, tag="mm")
            for nb in range(2):
                s = slice(512 * nb, 512 * (nb + 1))
                for g in range(4):
                    nc.tensor.matmul(pm[:, s], wt[:, g, :, 128 * mm:128 * (mm + 1)],
                                     h1[g][:, :, s], start=(g == 0), stop=(g == 3),
                                     perf_mode=DR)
            qk = qkp.tile([128, T], BF16, tag="qk")
            nc.scalar.activation(qk, pm, AF.Identity, bias=b_qkT[:, m:m + 1])
            qkT.append(qk)
    # vT token-major, DR pairs [128, 2, HL, HD+1]
    wv_blk = [load_w(io["w_v"], FP8, b, wst) for b in range(2)]
    vdr = [vtp.tile([128, 2, HL, HD + 1], FP8, tag="vt", name=f"vdr{i}")
           for i in range(4)]
    for m in range(NCH):
        pv = ps_aux.tile([128, DL], FP32, tag="aux")
        for b in range(2):
            for g in range(4):
                nc.tensor.matmul(pv[:, 256 * b:256 * (b + 1)],
                                 h1[g][:, :, 128 * m:128 * (m + 1)],
                                 wv_blk[b][:, g, :, :],
                                 start=(g == 0), stop=(g == 3), perf_mode=DR)
        nc.vector.tensor_tensor(vdr[m // 2][:, m % 2, :, 0:HD],
                                pv.rearrange("p (a b) -> p a b", a=HL),
                                bv_b.rearrange("p (a b) -> p a b", a=HL), op=ALU.add)
        nc.sync.dma_start(out=vdr[m // 2][:, m % 2, :, HD:HD + 1],
                          in_=io["ones_8"].rearrange("p (j o) -> p j o", o=1))

    # ---------------- cross-attn context (independent of x; fills stalls) ----
    teT = sb.tile([128, 3, 2, TLP], FP8, tag="teT")
    nc.sync.dma_start(out=teT, in_=io["teT"])
    b_ctxT = sb.tile([128, 8], FP32, tag="b_ctxT")
    nc.sync.dma_start(out=b_ctxT, in_=io["b_ctxT"])
    ctx4 = [pools["ctxp"].tile([128, 2, TLP], FP8, tag="ctxT", name=f"ctx{g}")
            for g in range(4)]
    for mb in range(4):
        wt = load_w(io["w_ctx"], FP8, mb, wst)
        for mm in range(2):
            j = 2 * mb + mm
            pc = ps_aux.tile([128, TLP], FP32, tag="aux")
            for g in range(3):
                nc.tensor.matmul(pc, wt[:, g, :, 128 * mm:128 * (mm + 1)],
                                 teT[:, g, :, :], start=(g == 0), stop=(g == 2),
                                 perf_mode=DR)
            nc.vector.tensor_scalar(ctx4[j // 2][:, j % 2, :], pc,
                                    b_ctxT[:, j:j + 1], None, op0=ALU.add)
    # k_ctx feature-major [128, TLP] bf16 x4; v_ctx [80, HL, HD+1] bf16
    kcT = []
    for mb in range(2):
        wt = load_w(io["w_k"], FP8, mb, wst)
        for mm in range(2):
            m = 2 * mb + mm
            pk = ps_aux.tile([128, TLP], FP32, tag="aux")
            for g in range(4):
                nc.tensor.matmul(pk, wt[:, g, :, 128 * mm:128 * (mm + 1)],
                                 ctx4[g], start=(g == 0), stop=(g == 3), perf_mode=DR)
            kt = qkp.tile([128, TLP], BF16, tag="qkc")
            nc.scalar.copy(kt, pk)
            kcT.append(kt)
    wvc_blk = [load_w(io["w_vc"], FP8, b, wst) for b in range(2)]
    pvc = ps_aux.tile([TLP, DL], FP32, tag="aux")
    for b in range(2):
        for g in range(4):
            nc.tensor.matmul(pvc[:, 256 * b:256 * (b + 1)], ctx4[g],
                             wvc_blk[b][:, g, :, :], start=(g == 0), stop=(g == 3),
                             perf_mode=DR)
    vc = vtp.tile([128, HL, HD + 1], BF16, tag="vtc")
    nc.vector.memset(vc, 0.0)
    nc.vector.tensor_copy(vc[0:TL, :, 0:HD], pvc[0:TL].rearrange("p (a b) -> p a b", a=HL))
    nc.sync.dma_start(out=vc[0:TL, :, HD:HD + 1],
                      in_=io["ones_b"][0:TL, :].rearrange("p (j o) -> p j o", o=1))

    # ---------------- attention ----------------
    def attention_self(q_tiles, k_tiles):
        at = [atp.tile([128, 2, T], FP8, tag="at", name=f"at{i}") for i in range(2)]
        for hpair in range(4):
            rcp2 = rcpp.tile([34, T], FP32R, tag="rcp2", name=f"rcp{hpair}")
            pos = []
            for e in range(2):
                h = 2 * hpair + e
                ti, off = h // 2, 64 * (h % 2)
                q_ap = q_tiles[ti][off:off + 64, :]
                pdr = [pp.tile([128, 2, T], FP8, tag="p", name=f"p{h}_{i}")
                       for i in range(4)]
                for m in range(8):
                    ps_sc = ps_mm.tile([128, T], FP32, tag="mm")
                    k_ap = k_tiles[ti][off:off + 64, 128 * m:128 * (m + 1)]
                    for nb in range(2):
                        s = slice(512 * nb, 512 * (nb + 1))
                        nc.tensor.matmul(ps_sc[:, s], k_ap, q_ap[:, s],
                                         start=True, stop=True)
                    nc.scalar.activation(pdr[m // 2][:, m % 2, :], ps_sc, AF.Exp,
                                         bias=expb_t, scale=float(HD) ** -0.5)
                po = ps_aux.tile([HD + 1, T], FP32, tag="aux", name=f"po{h}")
                for mp in range(4):
                    for nb in range(2):
                        s = slice(512 * nb, 512 * (nb + 1))
                        nc.tensor.matmul(po[:, s], vdr[mp][:, :, h, :],
                                         pdr[mp][:, :, s], start=(mp == 0),
                                         stop=(mp == 3), perf_mode=DR,
                                         skip_group_check=True)
                with nc.allow_low_precision(reason="fp32r rcp feeds broadcast"):
                    nc.vector.reciprocal(rcp2[32 + e:33 + e, :], po[HD:HD + 1, :])
                pos.append(po)
            pb = ps_mm.tile([128, T], FP32, tag="mm", name=f"pb{hpair}")
            for nb in range(2):
                s = slice(512 * nb, 512 * (nb + 1))
                nc.tensor.matmul(pb[:, s], ones2[32:34, :], rcp2[32:34, s], start=True, stop=True)
            rc = scr.tile([128, T], FP32R, tag="t1")
            nc.vector.tensor_copy(rc, pb)
            for e in range(2):
                h = 2 * hpair + e
                ti = h // 2
                off = 64 * (h % 2)
                nc.vector.tensor_tensor(at[ti // 2][off:off + 64, ti % 2, :],
                                        pos[e][0:HD, :], rc[off:off + 64, :],
                                        op=ALU.mult)
        return at

    def attention_cross(q_tiles, k_tiles):
        at = [atp.tile([128, 2, T], FP8, tag="at", name=f"atc{i}") for i in range(2)]
        for hpair in range(4):
            rcp2 = rcpp.tile([34, T], FP32R, tag="rcp2", name=f"rcpc{hpair}")
            pos = []
            for e in range(2):
                h = 2 * hpair + e
                ti, off = h // 2, 64 * (h % 2)
                ps_sc = ps_mm.tile([128, T], FP32, tag="mm")
                k_ap = k_tiles[ti][off:off + 64, :]
                for nb in range(2):
                    s = slice(512 * nb, 512 * (nb + 1))
                    nc.tensor.matmul(ps_sc[:TLP, s], k_ap, q_tiles[ti][off:off + 64, s],
                                     start=True, stop=True)
                pt = pp.tile([128, T], BF16, tag="p", name=f"pc{h}")
                nc.scalar.activation(pt[:TLP, :], ps_sc[:TLP, :], AF.Exp,
                                     bias=expb_t[:TLP], scale=float(HD) ** -0.5)
                po = ps_aux.tile([HD + 1, T], FP32, tag="aux", name=f"poc{h}")
                for nb in range(2):
                    s = slice(512 * nb, 512 * (nb + 1))
                    nc.tensor.matmul(po[:, s], vc[0:TLP, h, :], pt[0:TLP, s],
                                     start=True, stop=True)
                with nc.allow_low_precision(reason="fp32r rcp feeds broadcast"):
                    nc.vector.reciprocal(rcp2[32 + e:33 + e, :], po[HD:HD + 1, :])
                pos.append(po)
            pb = ps_mm.tile([128, T], FP32, tag="mm", name=f"pbc{hpair}")
            for nb in range(2):
                s = slice(512 * nb, 512 * (nb + 1))
                nc.tensor.matmul(pb[:, s], ones2[32:34, :], rcp2[32:34, s], start=True, stop=True)
            rc = scr.tile([128, T], FP32R, tag="t1")
            nc.vector.tensor_copy(rc, pb)
            for e in range(2):
                h = 2 * hpair + e
                ti = h // 2
                off = 64 * (h % 2)
                nc.vector.tensor_tensor(at[ti // 2][off:off + 64, ti % 2, :],
                                        pos[e][0:HD, :], rc[off:off + 64, :],
                                        op=ALU.mult)
        return at

    # row-parallel matmul + bf16 delta AllReduce + local residual add
    def row_parallel_reduce(w_name, bT_name, at, ar_tag):
        bT = sb.tile([128, 8], FP32, tag=bT_name)
        nc.sync.dma_start(out=bT, in_=io[bT_name])
        ar_in = dram.tile([D, T], BF16, tag=ar_tag + "_i")
        ar_out = dram.tile([D, T], BF16, tag=ar_tag + "_o")
        for mb in range(2):
            wt = load_w(io[w_name], FP8, mb, wst)
            for mm in range(4):
                m = 4 * mb + mm
                pm = ps_mm.tile([128, T], FP32, tag="mm")
                for nb in range(2):
                    s = slice(512 * nb, 512 * (nb + 1))
                    for g in range(2):
                        nc.tensor.matmul(pm[:, s], wt[:, g, :, 128 * mm:128 * (mm + 1)],
                                         at[g][:, :, s], start=(g == 0), stop=(g == 1),
                                         perf_mode=DR)
                dd = scr.tile([128, T], BF16, tag="t1")
                nc.scalar.activation(dd, pm, AF.Identity, bias=bT[:, m:m + 1])
                nc.gpsimd.dma_start(out=ar_in[128 * m:128 * (m + 1), :], in_=dd)
        for ch in range(4):
            sl = slice(256 * ch, 256 * (ch + 1))
            if nocc:
                nc.gpsimd.dma_start(out=ar_out[sl, :], in_=ar_in[sl, :])
            else:
                nc.gpsimd.collective_compute(
                    "AllReduce", ALU.add, replica_groups=PAIR_GROUPS,
                    ins=[ar_in[sl, :].opt()], outs=[ar_out[sl, :].opt()])
        for m in range(NCH):
            dt = ddp.tile([128, T], BF16, tag="dt")
            nc.gpsimd.dma_start(out=dt, in_=ar_out[128 * m:128 * (m + 1), :])
            nc.vector.tensor_tensor(x_tiles[m], x_tiles[m], dt, op=ALU.add)

    at1 = attention_self(qkT[0:4], qkT[4:8])
    row_parallel_reduce("w_proj", "b_projT", at1, "arp")

    # ---------------- Stage 4: cross-attention ----------------
    h2 = [hp.tile([128, 2, T], FP8, tag="h", name=f"h2_{g}") for g in range(4)]

    def write_h2(j, t1, S):
        nc.vector.tensor_tensor(h2[j // 2][:, j % 2, :], t1, S, op=ALU.subtract)

    layer_norm_mod(x_tiles, 1, write_h2)

    q2T = []
    for mb in range(2):
        wt = load_w(io["w_q"], FP8, mb, wst)
        for mm in range(2):
            m = 2 * mb + mm
            pm = ps_mm.tile([128, T], FP32, tag="mm")
            for nb in range(2):
                s = slice(512 * nb, 512 * (nb + 1))
                for g in range(4):
                    nc.tensor.matmul(pm[:, s], wt[:, g, :, 128 * mm:128 * (mm + 1)],
                                     h2[g][:, :, s], start=(g == 0), stop=(g == 3),
                                     perf_mode=DR)
            qt = qkp.tile([128, T], BF16, tag="qk")
            nc.scalar.copy(qt, pm)
            q2T.append(qt)

    at2 = attention_cross(q2T, kcT)
    row_parallel_reduce("w_out", "b_outT", at2, "aro")

    # ---------------- Stage 5: MLP (bf16) ----------------
    h3 = []
    b_fc1T = sb.tile([128, 16], FP32, tag="b_fc1T")
    nc.sync.dma_start(out=b_fc1T, in_=io["b_fc1T"])
    b_fc2T = sb.tile([128, 8], FP32, tag="b_fc2T")
    nc.sync.dma_start(out=b_fc2T, in_=io["b_fc2T"])

    def write_h3(j, t1, S):
        ht = h3p.tile([128, T], BF16, tag="h3")
        nc.vector.tensor_tensor(ht, t1, S, op=ALU.subtract)
        h3.append(ht)

    layer_norm_mod(x_tiles, 2, write_h3)

    for tb in range(2):
        s = slice(512 * tb, 512 * (tb + 1))
        hid = []
        for mb in range(8):
            wt = load_w(io["w_fc1"], BF16, mb, wst2)
            for mm in range(2):
                m = 2 * mb + mm
                pm = ps_aux.tile([128, 512], FP32, tag="aux")
                for k in range(NCH):
                    nc.tensor.matmul(pm, wt[:, k, 128 * mm:128 * (mm + 1)], h3[k][:, s],
                                     start=(k == 0), stop=(k == NCH - 1))
                ht = hidp.tile([128, 512], BF16, tag="hid")
                nc.scalar.activation(ht, pm, AF.Gelu, bias=b_fc1T[:, m:m + 1])
                hid.append(ht)
        for mb in range(4):
            wt2 = load_w(io["w_fc2"], BF16, mb, wst3)
            for mm in range(2):
                m = 2 * mb + mm
                pm = ps_mm.tile([128, 512], FP32, tag="mm")
                for k in range(FFL // 128):
                    nc.tensor.matmul(pm, wt2[:, k, 128 * mm:128 * (mm + 1)], hid[k],
                                     start=(k == 0), stop=(k == FFL // 128 - 1))
                ot = scr.tile([128, 512], FP32R, tag="t1")
                nc.vector.tensor_scalar(ot, pm, b_fc2T[:, m:m + 1], None, op0=ALU.add)
                nc.vector.tensor_tensor(ot, ot, x_tiles[m][:, s], op=ALU.add)
                nc.sync.dma_start(out=io["out_xT"][128 * m:128 * (m + 1), s], in_=ot)


def build(nocc=False):
    nc = bacc.Bacc("TRN2", target_bir_lowering=False, debug=False,
                   num_devices=1 if nocc else 8)
    io = _declare(nc)
    with tile.TileContext(nc) as tc:
        import contextlib
        with contextlib.ExitStack() as ctx:
            def pool(name, bufs, space="SBUF"):
                return ctx.enter_context(tc.tile_pool(name=name, bufs=bufs, space=space))
            pools = {
                "sb": pool("sb", 1),
                "xp": pool("xp", 8),
                "hp": pool("hp", 4),
                "h3p": pool("h3p", 8),
                "qkp": pool("qkp", 8),
                "vtp": pool("vtp", 5),
                "atp": pool("atp", 2),
                "pp": pool("pp", 8),
                "wadap": pool("wadap", 2),
                "wst": pool("wst", 4),
                "wst2": pool("wst2", 2),
                "wst3": pool("wst3", 2),
                "hidp": pool("hidp", 16),
                "xsqp": pool("xsqp", 1),
                "scr": pool("scr", 2),
                "rcpp": pool("rcpp", 2),
                "ddp": pool("ddp", 2),
                "ctxp": pool("ctxp", 4),
                "vecp": pool("vecp", 1),
                "ps_mm": pool("ps_mm", 2, "PSUM"),
                "ps_aux": pool("ps_aux", 2, "PSUM"),
                "dram": pool("dram", 1, "DRAM"),
            }
            _emit(tc, io, pools, nocc=nocc)
    nc.compile()
    return nc


def pretile(w, mblk):
    """[K, M] -> [M//mblk, 128, K//128, mblk]"""
    K, M = w.shape
    v = w.reshape(K // 128, 128, M // mblk, mblk).transpose(2, 1, 0, 3)
    return np.ascontiguousarray(v)


def pretile_dr(w, mblk):
    """[K, M] -> [M//mblk, 128, K//256, 2, mblk] (DoubleRow k-pairs)"""
    K, M = w.shape
    v = w.reshape(K // 256, 2, 128, M // mblk, mblk).transpose(3, 2, 0, 1, 4)
    return np.ascontiguousarray(v)


def shard_inputs(inputs):
    f32 = np.float32
    bf16 = ml_dtypes.bfloat16
    f8 = ml_dtypes.float8_e4m3
    x = np.asarray(inputs["x"], f32)
    c = np.asarray(inputs["c"], f32)
    te = np.asarray(inputs["text_embed"], f32)
    W_ada, b_ada = np.asarray(inputs["W_ada"], f32), np.asarray(inputs["b_ada"], f32)
    W_qkv, b_qkv = np.asarray(inputs["W_qkv"], f32), np.asarray(inputs["b_qkv"], f32)
    W_proj, b_proj = np.asarray(inputs["W_proj"], f32), np.asarray(inputs["b_proj"], f32)
    W_ctx, b_ctx = np.asarray(inputs["W_ctx"], f32), np.asarray(inputs["b_ctx"], f32)
    W_q, W_k, W_v = (np.asarray(inputs[k], f32) for k in ("W_q", "W_k", "W_v"))
    W_out, b_out = np.asarray(inputs["W_out"], f32), np.asarray(inputs["b_out"], f32)
    W_fc1, b_fc1 = np.asarray(inputs["W_fc1"], f32), np.asarray(inputs["b_fc1"], f32)
    W_fc2, b_fc2 = np.asarray(inputs["W_fc2"], f32), np.asarray(inputs["b_fc2"], f32)

    cT = np.ascontiguousarray(c.T.reshape(8, 128, B).transpose(1, 0, 2))
    teTp = np.pad(te.transpose(0, 2, 1), ((0, 0), (0, 0), (0, TLP - TL)))

    maps = []
    for core in range(8):
        b, hf = core // 2, core % 2
        sl = slice(DL * hf, DL * (hf + 1))
        half = (lambda a: a) if hf == 0 else (lambda a: np.zeros_like(a))
        qs = slice(DL * hf, DL * (hf + 1))
        ks_ = slice(D + DL * hf, D + DL * (hf + 1))
        vs = slice(2 * D + DL * hf, 2 * D + DL * (hf + 1))
        # adaLN interleaved col slice: chunks cc with cc % 8 == core
        acols = np.concatenate([np.arange(128 * (8 * t + core), 128 * (8 * t + core) + 128)
                                for t in range(6)])
        m = {
            "xT": np.ascontiguousarray(x[b].T),
            "cT": cT.astype(bf16),
            "teT": np.ascontiguousarray(
                teTp[b].reshape(3, 2, 128, TLP).transpose(2, 0, 1, 3)).astype(f8),
            "w_ada": pretile(W_ada[:, acols].astype(bf16), 128),
            "b_adaT": np.ascontiguousarray(b_ada[acols].reshape(6, 128).T),
            "w_qk": pretile_dr(np.concatenate(
                [W_qkv[:, qs], W_qkv[:, ks_]], axis=1).astype(f8), 256),
            "b_qkT": np.ascontiguousarray(
                np.concatenate([b_qkv[qs], b_qkv[ks_]]).reshape(8, 128).T),
            "w_v": pretile_dr(W_qkv[:, vs].astype(f8), 256),
            "b_v": b_qkv[vs][None, :].copy(),
            "w_proj": pretile_dr(W_proj[sl, :].astype(f8), 512),
            "b_projT": np.ascontiguousarray(half(b_proj).reshape(8, 128).T),
            "w_ctx": pretile_dr(W_ctx.astype(f8), 256),
            "b_ctxT": np.ascontiguousarray(b_ctx.reshape(8, 128).T),
            "w_q": pretile_dr(W_q[:, sl].astype(f8), 256),
            "w_k": pretile_dr(W_k[:, sl].astype(f8), 256),
            "w_vc": pretile_dr(W_v[:, sl].astype(f8), 256),
            "w_out": pretile_dr(W_out[sl, :].astype(f8), 512),
            "b_outT": np.ascontiguousarray(half(b_out).reshape(8, 128).T),
            "w_fc1": pretile(W_fc1[:, FFL * hf:FFL * (hf + 1)].astype(bf16), 256),
            "b_fc1T": np.ascontiguousarray(
                b_fc1[FFL * hf:FFL * (hf + 1)].reshape(16, 128).T),
            "w_fc2": pretile(W_fc2[FFL * hf:FFL * (hf + 1), :].astype(bf16), 256),
            "b_fc2T": np.ascontiguousarray(half(b_fc2).reshape(8, 128).T),
            "ones_r": np.ones((128, 2), f32),
            "ones2": np.ascontiguousarray(np.concatenate(
                [np.ones((1, 128), f32), np.zeros((31, 128), f32),
                 np.repeat(np.eye(2, dtype=f32), 64, axis=1)])),
            "ones_8": np.ones((128, 8), f8),
            "ones_b": np.ones((128, 8), bf16),
        }
        maps.append(m)
    return maps


_NC_CACHE = None


def kernel(**inputs):
    global _NC_CACHE
    if _NC_CACHE is None:
        _NC_CACHE = build()
    in_maps = shard_inputs(inputs)
    res = run_bass_kernel_spmd(_NC_CACHE, in_maps, core_ids=list(range(8)))
    out = np.empty((B, N, D), np.float32)
    for b in range(B):
        p0 = res.results[2 * b]["out_xT"]
        p1 = res.results[2 * b + 1]["out_xT"]
        out[b] = (p0.astype(np.float32) + p1.astype(np.float32)).T
    return out
